# revision 42
# baseline (speedup 1.0000x reference)
"""Dynamic per-sample 3D Gaussian blur on 8 NeuronCores.

Sharding: pure data parallelism over (batch=4) x (channel=2) -> 8 cores,
one [160,160,160] volume per core. Per core the separable blur runs as
three banded-matmul passes on the TensorEngine (conv axis contracted on
partitions); the stationary operand is the data chunk so each pass also
rotates the layout for the next axis:

  pass1 (D):  x0[d', (h,w)] -> psum[h-chunk, d]  @ fixed w -> x1[h', (w,d)]
  pass2 (H):  x1[h', (w,d)] -> psum[w-chunk, h]  @ fixed d -> x2[w', (d,h)]
  pass3 (W):  x2[w', (d,h)] -> psum[(d,h)-chunk, w] -> staging -> HBM

The 160-long conv axis is split 128+32; outputs whose 13-tap window
crosses the split accumulate a second small matmul into the same PSUM
bank (per-element has_written semantics). Three conv-axis-tail tensors
share one [128, S] tile at partition ranges [0:32)/[32:64)/[64:96).
PSUM tiles batch 3 chunks (480 f32 cols, one bank) so each psum->SBUF
copy amortizes the ~125-170ns PSUM access latency; copies alternate
between VectorE and ScalarE; the input f32->bf16 cast is spread over
GpSimd/Vector/Scalar. All copy destinations are contiguous.
Data and band matrices are bf16, PSUM fp32; gaussians are computed on
host in fp32 exactly as the reference.
"""

from contextlib import ExitStack

import numpy as np
import ml_dtypes

import concourse.bass as bass
import concourse.tile as tile
from concourse import bacc, mybir
from concourse.bass_utils import run_bass_kernel_spmd

N = 160            # cube edge
S = N * N          # 25600 spatial positions per pass
NB = 13            # gaussian window
HALF = 6
A_N = 134          # big block out-cols [0, 134): windows within rows [0,128)
B_C0 = 122         # small block out-cols [122, 160): windows touching rows [128,160)
B_N = N - B_C0     # 38
GB_C0 = 3 * A_N    # col offset of the gb blocks in the packed G tile
G_COLS = 3 * A_N + B_N  # 440
EPS = 1e-7
GRP = 9            # psum chunks per pass-3 tile (3 banks: 3*160 f32/bank)
GA = 8             # pa chunks per group in passes 1/2 (20 groups of 8)
PBW = 4            # conv columns batched per tail-row stationary
BANK = 512         # psum bank capacity in f32
PBG = 3            # psum chunks per bank

BF16 = ml_dtypes.bfloat16
F32 = mybir.dt.float32
BF = mybir.dt.bfloat16

_PROGRAM = None


def _gaussian_1d(sigma):
    # fp32 replica of reference._gaussian_1d for a single sigma
    loc = (np.arange(NB, dtype=np.float32) - np.float32((NB - 1) / 2.0))
    s = np.float32(sigma)
    g = np.exp(-(loc * loc) / (2.0 * s * s + np.float32(EPS))
               - np.log(np.sqrt(np.float32(2.0 * np.pi)) * s + np.float32(EPS)))
    g = g.astype(np.float32)
    return g / g.sum(dtype=np.float32)


def _band(g):
    # T[r, c] = g[r - c + HALF] on the band, zero elsewhere ('SAME' zero pad)
    t = np.zeros((N, N), np.float32)
    for k in range(NB):
        off = k - HALF  # r = c + off
        c0 = max(0, -off)
        c1 = min(N, N - off)
        idx = np.arange(c0, c1)
        t[idx + off, idx] = g[k]
    return t


def _gpack(sigma_row):
    """[128, G_COLS] bf16: cols [p*134,(p+1)*134) = T_p[0:128, 0:134];
    cols [402:440) partitions [32p, 32p+32) = T_p[128:160, 122:160)."""
    out = np.zeros((128, G_COLS), np.float32)
    for p in range(3):
        t = _band(_gaussian_1d(sigma_row[p]))
        out[:, p * A_N:(p + 1) * A_N] = t[0:128, 0:A_N]
        out[32 * p:32 * (p + 1), GB_C0:G_COLS] = t[128:N, B_C0:N]
    return out.astype(BF16)


def _build_kernel(ctx, tc, x_in, g_in, y_out):
    nc = tc.nc

    gpool = ctx.enter_context(tc.tile_pool(name="g", bufs=1))
    big = ctx.enter_context(tc.tile_pool(name="big", bufs=1))
    tpk = ctx.enter_context(tc.tile_pool(name="tpk", bufs=1))
    tmp32 = ctx.enter_context(tc.tile_pool(name="tmp32", bufs=3))
    stage = ctx.enter_context(tc.tile_pool(name="stage", bufs=3))
    ps1 = ctx.enter_context(tc.tile_pool(name="ps1", bufs=2, space="PSUM"))
    ps2 = ctx.enter_context(tc.tile_pool(name="ps2", bufs=2, space="PSUM"))

    gtile = gpool.tile([128, G_COLS], BF)
    nc.sync.dma_start(gtile[:], g_in)

    def ga(p):  # [128, 134] base partition 0
        return gtile[:, (p - 1) * A_N:p * A_N]

    def gb(p):  # [32, 38] base partition 32*(p-1)
        return gtile[32 * (p - 1):32 * p, GB_C0:G_COLS]

    # persistent volume tiles; tails: [0:32) x0, [32:64) x1, [64:96) x2
    x0t1 = big.tile([128, S], BF, tag="sA")
    x1t1 = big.tile([128, S], BF, tag="sB")
    tails = big.tile([128, S], BF, tag="sT")
    # h-tail block (h 128:160 rows of the input) in its own tile so the
    # pass-1 pa wait covers only the h-main DMAs
    TP = 32 * N  # 5120
    xht = tpk.tile([128, TP], BF, tag="xht")

    # ---- input arrives bf16 from host: DMA straight into the volume
    # tiles, no staging/cast. Load order = consumption order: the h-main
    # block (pass-1 pa-big), d-tail rows (pa-small), then the h-tail
    # block (only the repack/pb path needs it, which runs LAG groups
    # behind pa).
    # one HWDGE ring sustains only ~200-260 GB/s, so spread the load:
    # h-main (gates the first matmul) split 60/40 across the two HWDGE
    # rings, with d-tail (needed by pa-small immediately) leading the
    # lighter ring and xht (repack/pb input, needed LAG groups later) on
    # the GpSimd SWDGE ring
    HM = 128 * N  # 20480
    H1 = 12800
    nc.sync.dma_start(x0t1[:, 0:H1], x_in[0:128, 0:H1])
    nc.scalar.dma_start(tails[0:32, :], x_in[128:160, :])
    nc.scalar.dma_start(x0t1[:, H1:HM], x_in[0:128, H1:HM])
    nc.gpsimd.dma_start(xht[:], x_in[0:128, HM:S])

    # packed tail-row stationaries: tm[d-part, c*32 + t] holds the
    # conv-output-row tail (rows 128:160 of the chunk axis) c-major, so a
    # PBW-wide group of columns is one contiguous 128-col LDWEIGHTS; tt
    # is the same packing of the conv-input tail rows (K=32 operand),
    # pass 1 at partitions [0:32), pass 2 at [32:64). tm is reused
    # across passes (dead once the pass's last pb issues).
    tt = tpk.tile([128, TP], BF, tag="tt")

    def repack(src_m, src_t, tm, tbase):
        # src views are [p, c, 32] (c = conv column, 32 = tail rows);
        # DVE/ACT strided gathers, chunked so the first pb group only
        # waits on chunk 0
        for i, c0 in enumerate(range(0, N, 40)):
            eng = nc.vector.tensor_copy if i % 2 == 0 else nc.scalar.copy
            eng(tm[:, c0 * 32:(c0 + 40) * 32], src_m[:, c0:c0 + 40, :])
        for i, c0 in enumerate(range(0, N, 80)):
            eng = nc.scalar.copy if i % 2 == 0 else nc.vector.tensor_copy
            eng(tt[tbase:tbase + 32, c0 * 32:(c0 + 80) * 32],
                src_t[:, c0:c0 + 80, :])

    LAG = 4  # pb groups issued this many groups behind pa

    def conv_pa(p, group):
        """pa: the [h-main 0:128] x [d' 0:160] block of one GA-column
        group; chunk j's psum lives at bank j//PBG, col (j%PBG)*160.
        Runs of same-shape stationaries let the PE weight ping-pong pull
        LDWEIGHTS ahead of in-flight matmuls; start=True clears the
        whole bank, so only the first matmul per bank sets it."""
        def off(j):
            return (j // PBG) * BANK + (j % PBG) * N

        cols, _, dst1, _, _ = group
        pa = ps1.tile([128, 3 * BANK], F32, tag="pa")
        for j, c1 in enumerate(cols):
            o = off(j)
            nc.tensor.matmul(pa[0:128, o:o + A_N], c1[0], ga(p),
                             start=(j % PBG == 0), stop=False,
                             skip_group_check=True)
        for j, c1 in enumerate(cols):
            o = off(j)
            nc.tensor.matmul(pa[0:128, o + B_C0:o + N], c1[1], gb(p),
                             start=False, stop=(j % PBG == PBG - 1 or
                                                j == GA - 1),
                             skip_group_check=True)
        pav = pa[:].rearrange("p (b c) -> p b c", c=BANK)
        nc.vector.tensor_copy(dst1[:, 0:2 * PBG * N],
                              pav[0:128, 0:2, 0:PBG * N])
        nc.scalar.copy(dst1[:, 2 * PBG * N:GA * N],
                       pa[0:128, 2 * BANK:2 * BANK + (GA - 2 * PBG) * N])

    UB = 2  # pb groups per unfold batch
    ust = {}

    def conv_pb(p, gi, group):
        """pb: batched tail-row chunks -- one contiguous 128-col packed
        stationary per PBW conv columns; psum partition block
        [32*qw, 32*qw+32) holds column c0 + q*PBW + qw. One drain per
        group to a bf16 stage frees the bank; every UB groups the stage
        is unfolded into the tail tile with 4 SBUF->SBUF DMAs on the ACT
        ring (no engine time, and no head-of-line with output DMAs)."""
        _, bcols, _, tdst, c0 = group
        pb = ps2.tile([128, PBG * N], F32, tag="pb")
        nq = GA // PBW
        for q, b1 in enumerate(bcols):
            nc.tensor.matmul(pb[0:128, q * N:q * N + A_N], b1[0], ga(p),
                             start=(q == 0), stop=False,
                             skip_group_check=True)
        for q, b1 in enumerate(bcols):
            nc.tensor.matmul(pb[0:128, q * N + B_C0:q * N + N], b1[1], gb(p),
                             start=False, stop=(q == nq - 1),
                             skip_group_check=True)
        b = gi % UB
        if b == 0:
            ust["stg"] = stage.tile([128, UB * nq * N], BF, tag="pbs",
                                    name=f"pbs_{p}_{gi}", uniquify=True)
        stg = ust["stg"]
        nc.vector.tensor_copy(stg[:, b * nq * N:(b + 1) * nq * N],
                              pb[0:128, 0:nq * N])
        if b == UB - 1:
            nm = UB * nq
            sv = stg[:].rearrange("p (m w) -> p m w", w=N)
            tv = (tdst[:, (c0 - (UB - 1) * GA) * N:(c0 + GA) * N]
                  .rearrange("p (m b w) -> p m b w", m=nm, b=PBW))
            for qw in range(PBW):
                nc.sync.dma_start(tv[:, :, qw, :],
                                  sv[32 * qw:32 * qw + 32, :, :])

    def conv_pass(p, groups, after=None):
        gs = list(groups)
        for i in range(len(gs) + LAG):
            if i < len(gs):
                conv_pa(p, gs[i])
            if i >= LAG:
                conv_pb(p, i - LAG, gs[i - LAG])
            if after is not None:
                after(i)

    # ---- pass 1: conv along d; chunks = h values at fixed w (strided
    # cols). pa-big reads x0t1 ([d, h-main 0:128 x w]); the h-tail rows
    # live in xht and only feed the packed pb stationaries.
    x0v1 = x0t1[:].rearrange("p (h w) -> p h w", w=N)
    x0v2 = tails[0:32].rearrange("p (h w) -> p h w", w=N)
    tm = tpk.tile([128, TP], BF, tag="tm")

    def pk_groups(v1, v2, tbase, dst, tdst):
        for c0 in range(0, N, GA):
            cols = [(v1[:, 0:128, c0 + j], v2[:, 0:128, c0 + j])
                    for j in range(GA)]
            bcols = [(tm[:, (c0 + q * PBW) * 32:(c0 + (q + 1) * PBW) * 32],
                      tt[tbase:tbase + 32,
                         (c0 + q * PBW) * 32:(c0 + (q + 1) * PBW) * 32])
                     for q in range(GA // PBW)]
            # next-pass free layout (c, ax'): c-group is contiguous
            yield (cols, bcols, dst[:, c0 * N:(c0 + GA) * N], tdst, c0)

    groups1 = list(pk_groups(x0v1, x0v2, 0, x1t1, tails[32:64]))

    # x2t1 reuses x0's slot (x0 fully consumed by pass 1)
    x2t1 = big.tile([128, S], BF, tag="sA")

    # ---- pass 2: conv along h; chunks = w values at fixed d (strided
    # cols); tm is re-packed in place for pass 2 (same tile, WAR at the
    # pass boundary)
    x1v1 = x1t1[:].rearrange("p (w d) -> p w d", d=N)
    x1v2 = tails[32:64].rearrange("p (w d) -> p w d", d=N)

    def p2_groups():
        return pk_groups(x1v1, x1v2, 32, x2t1, tails[64:96])

    # ---- pass 3: conv along w; chunks = contiguous (d,h) blocks; to HBM.
    # Pass-3 groups are interleaved into pass-2's program order as soon as
    # their x2 columns are complete, so the output DMA streams during the
    # PE-bound middle instead of piling up in a tail.
    x2v2 = tails[64:96]
    NK = S // 128  # 200

    def off3(j):
        return (j // PBG) * BANK + (j % PBG) * N

    def p3_group(k0):
        glen = min(GRP, NK - k0)
        ps = ps1.tile([128, 3 * BANK], F32, tag="pa", name=f"p3_{k0}")
        for j in range(glen):
            o = off3(j)
            c = (k0 + j) * 128
            nc.tensor.matmul(ps[:, o:o + A_N], x2t1[:, c:c + 128], ga(3),
                             start=(j % PBG == 0), stop=False,
                             skip_group_check=True)
        for j in range(glen):
            o = off3(j)
            c = (k0 + j) * 128
            nc.tensor.matmul(ps[:, o + B_C0:o + N], x2v2[:, c:c + 128], gb(3),
                             start=False,
                             stop=(j % PBG == PBG - 1 or j == glen - 1),
                             skip_group_check=True)
        st = tmp32.tile([128, GRP * N], BF, tag="t", name=f"st_{k0}")
        eng_copy = (nc.vector.tensor_copy if (k0 // GRP) % 2 == 0
                    else nc.scalar.copy)
        if glen == GRP:
            psv = ps[:].rearrange("p (b c) -> p b c", c=BANK)
            eng_copy(st[:, 0:GRP * N], psv[0:128, 0:GRP // PBG, 0:PBG * N])
        else:
            for q0 in range(0, glen, PBG):
                qn = min(PBG, glen - q0) * N
                eng_copy(st[:, q0 * N:q0 * N + qn],
                         ps[0:128, (q0 // PBG) * BANK:(q0 // PBG) * BANK + qn])
        eng_dma = nc.sync if (k0 // GRP) % 2 == 0 else nc.scalar
        eng_dma.dma_start(y_out[0:128, k0 * N:(k0 + glen) * N],
                          st[:, 0:glen * N])

    state = {"k0": 0}

    def p3_after(i):
        # x2 tail rows land only when a pb unfold batch completes: after
        # pb group i-LAG, (i+1-LAG)//UB batches of UB*GA cols are out
        done_d = min(N, UB * GA * ((i + 1 - LAG) // UB))
        while (state["k0"] < NK and
               (min(state["k0"] + GRP, NK) * 128 - 1) // N < done_d):
            p3_group(state["k0"])
            state["k0"] += GRP

    # ---- interleaved schedule. Pass 2's first LAG pa groups depend only
    # on pass 1's first 16 groups (w < 128), so they fill the PE while
    # pass 1's trailing pb groups and the pass-2 repack drain.
    NG = len(groups1)  # 20
    groups2 = list(p2_groups())
    for i in range(NG):
        conv_pa(1, groups1[i])
        if i == 1:
            repack(xht[:].rearrange("p (h w) -> p w h", w=N),
                   tails[0:32].rearrange("p (h w) -> p w h", w=N)
                   [:, :, 128:160], tm, 0)
        if i >= LAG:
            conv_pb(1, i - LAG, groups1[i - LAG])
    for i in range(NG, NG + LAG):
        conv_pa(2, groups2[i - NG])
        conv_pb(1, i - LAG, groups1[i - LAG])
    repack(x1t1[:].rearrange("p (w d) -> p d w", d=N)[:, :, 128:160],
           tails[32:64].rearrange("p (w d) -> p d w", d=N)[:, :, 128:160],
           tm, 32)
    for i in range(LAG, NG + LAG):
        if i < NG:
            conv_pa(2, groups2[i])
        conv_pb(2, i - LAG, groups2[i - LAG])
        p3_after(i)
    while state["k0"] < NK:
        p3_group(state["k0"])
        state["k0"] += GRP


def _build_program():
    global _PROGRAM
    if _PROGRAM is not None:
        return _PROGRAM
    nc = bacc.Bacc("TRN2", target_bir_lowering=False, debug=False,
                   num_devices=8)
    x_in = nc.dram_tensor("x_in", [N, S], BF, kind="ExternalInput").ap()
    g_in = nc.dram_tensor("g_in", [128, G_COLS], BF, kind="ExternalInput").ap()
    # packed output: [p, (k, w)] bf16 so every partition's DMA run is
    # contiguous (host unshuffles (k p) -> spatial); f32 upcast on host
    y_out = nc.dram_tensor("y_out", [128, (S // 128) * N], BF,
                           kind="ExternalOutput").ap()
    with tile.TileContext(nc) as tc, ExitStack() as ctx:
        _build_kernel(ctx, tc, x_in, g_in, y_out)
    nc.compile()
    _PROGRAM = nc
    return nc


def _run(image, sigma, **spmd_kwargs):
    nc = _build_program()
    B, _, _, _, C = image.shape
    in_maps = []
    for core in range(8):
        b, c = divmod(core, C)
        vol = np.ascontiguousarray(image[b, :, :, :, c]).reshape(N, S)
        in_maps.append({"x_in": vol.astype(BF16), "g_in": _gpack(sigma[b])})
    res = run_bass_kernel_spmd(nc, in_maps, list(range(8)), **spmd_kwargs)
    out = np.empty((B, N, N, N, C), np.float32)
    for core in range(8):
        b, c = divmod(core, C)
        y = res.results[core]["y_out"].astype(np.float32)
        # y[p, k*N+w] holds spatial row k*128+p
        y = y.reshape(128, S // 128, N).transpose(1, 0, 2)
        out[b, :, :, :, c] = y.reshape(N, N, N)
    return out, res


def kernel(image, sigma):
    image = np.asarray(image, dtype=np.float32)
    sigma = np.asarray(sigma, dtype=np.float32)
    out, _ = _run(image, sigma)
    return out

